# revision 79
# baseline (speedup 1.0000x reference)
"""Trainium2 Bass kernel for nn_DotProductAttention_79929341378723.

Computation (per batch b, sharded batch-parallel over 8 NeuronCores):
    q   = gelu(query @ Wq + bq)          [LQ, H]
    v   = gelu(value @ Wv + bv)          [LV, H]
    s   = q @ v.T                        [LQ, LV]
    a   = softmax(s, axis=-1)            [LQ, LV]   (output)
    ctx = a @ v                          [LQ, H]
    out = gelu(ctx) @ Wo + bo            [LQ, D]    (output)

Design notes:
  - Scores are computed TWICE on the PE in the two orientations needed:
    scoreT [k, m] feeds exp -> expT tiles used directly as the moving operand
    of the ctx matmul (contraction k on partitions, no transposes of the
    16.7M-element attention matrix); score [m, k] feeds exp(+row sums via
    accum_out) -> normalized attn written straight out to HBM in its native
    layout (2MB contiguous DMAs).
  - All large matmuls run in float32r (TF32-like, 1 cycle/row at N=512);
    operands are rounded to f32r by their producing ACT/DVE instruction.
  - Softmax skips max-subtraction: |score| < ~30 for this distribution, so
    exp() cannot overflow fp32 and max-subtraction changes nothing
    numerically (it divides num/denom by the same constant).
  - The only cross-orientation data needed are the row sums: computed in the
    [m, k] pass via activation accum_out; broadcast along partitions for the
    ctx normalization via per-m-tile PE transpose + rank-1 ones matmul.
"""

import numpy as np
from contextlib import ExitStack

import concourse.bass as bass
import concourse.tile as tile
import concourse.mybir as mybir
from concourse import bacc
from concourse.bass import ts
from concourse.bass_utils import run_bass_kernel_spmd
from concourse.masks import make_identity

F32 = mybir.dt.float32
F32R = mybir.dt.float32r
AF = mybir.ActivationFunctionType
AX = mybir.AxisListType

B, LQ, LV = 8, 4096, 4096
D_OUT, D_HID = 128, 128
P = 128
NT_Q = LQ // P          # 32 query tiles
NT_K = LV // P          # 32 key/value tiles
MC = 1024               # pass-1 m-chunk width (exp free-dim)
NCH = LQ // MC          # 4 outer chunks
KC = 1024               # pass-2 k-chunk width (exp free-dim)
NKC = LV // KC          # 4 k-chunks per m-tile


def _body(tc):
    nc = tc.nc
    query = nc.dram_tensor("query", [LQ, D_OUT], F32, kind="ExternalInput").ap()
    value = nc.dram_tensor("value", [LV, D_HID], F32, kind="ExternalInput").ap()
    Wq = nc.dram_tensor("Wq", [D_OUT, D_HID], F32, kind="ExternalInput").ap()
    bq = nc.dram_tensor("bq", [D_HID], F32, kind="ExternalInput").ap()
    Wv = nc.dram_tensor("Wv", [D_HID, D_HID], F32, kind="ExternalInput").ap()
    bv = nc.dram_tensor("bv", [D_HID], F32, kind="ExternalInput").ap()
    Wo = nc.dram_tensor("Wo", [D_HID, D_OUT], F32, kind="ExternalInput").ap()
    bo = nc.dram_tensor("bo", [D_OUT], F32, kind="ExternalInput").ap()
    out = nc.dram_tensor("out", [LQ, D_OUT], F32, kind="ExternalOutput").ap()
    attn = nc.dram_tensor("attn", [LQ, LV], F32, kind="ExternalOutput").ap()

    attn_t_view = attn.rearrange("(t p) k -> t p k", p=P)

    with ExitStack() as ctx:
        const = ctx.enter_context(tc.tile_pool(name="const", bufs=1))
        stat = ctx.enter_context(tc.tile_pool(name="stat", bufs=1))

        ident = const.tile([P, P], F32, tag="ident")
        make_identity(nc, ident)

        # Weights: load fp32, round to f32r via DVE copy (matmul operands
        # must be written as f32r by their producer).
        wq_f = const.tile([P, P], F32, tag="wqf")
        wv_f = const.tile([P, P], F32, tag="wvf")
        wo_f = const.tile([P, P], F32, tag="wof")
        nc.sync.dma_start(out=wq_f, in_=Wq)
        nc.sync.dma_start(out=wv_f, in_=Wv)
        nc.sync.dma_start(out=wo_f, in_=Wo)
        Wq_r = const.tile([P, P], F32R, tag="wq")
        Wv_r = const.tile([P, P], F32R, tag="wv")
        Wo_r = const.tile([P, P], F32R, tag="wo")
        nc.vector.tensor_copy(Wq_r, wq_f)
        nc.vector.tensor_copy(Wv_r, wv_f)
        nc.vector.tensor_copy(Wo_r, wo_f)
        bq_t = const.tile([P, 1], F32, tag="bq")
        bv_t = const.tile([P, 1], F32, tag="bv")
        nc.sync.dma_start(out=bq_t, in_=bq.rearrange("(p o) -> p o", o=1))
        nc.sync.dma_start(out=bv_t, in_=bv.rearrange("(p o) -> p o", o=1))
        # prewarm the gelu act-table while the input DMAs stream
        warm = const.tile([P, 1], F32, tag="warm")
        nc.scalar.activation(warm, bq_t, AF.Gelu)
        # bo broadcast across partitions (DRE replication on SWDGE)
        bo_bc = const.tile([P, D_OUT], F32, tag="bo")
        bo_bcast_ap = bass.AP(tensor=bo.tensor, offset=bo.offset,
                              ap=[[0, P], [1, D_OUT]])
        nc.gpsimd.dma_start(out=bo_bc, in_=bo_bcast_ap)

        # Persistent activations (transposed layouts: [h, seq])
        q_actT = stat.tile([P, LQ], F32R, tag="q_actT")
        v_actT = stat.tile([P, LV], F32R, tag="v_actT")
        v_act = stat.tile([P, NT_K, D_HID], F32R, tag="v_act")  # [k, t, h]
        ctxT = stat.tile([P, LQ], F32, tag="ctxT")              # [h, m]
        ctxn = stat.tile([P, LQ], F32, tag="ctxn")              # normalized
        inv_sums = stat.tile([P, NT_Q], F32, tag="inv_sums")    # [m_in_tile, t]
        out_sb = stat.tile([P, NT_Q, D_OUT], F32, tag="out_sb")
        ones_f = stat.tile([1, P], F32, tag="ones_f")
        ones_r = stat.tile([1, P], F32R, tag="ones_r")

        def build_ones():
            nc.gpsimd.memset(ones_f, 1.0)
            nc.vector.tensor_copy(ones_r, ones_f)

        # ---------------- Phase 1: input transposes + q/v linears ----------
        # q and v interleave per-chunk so the first attention chunk's inputs
        # are ready as early as possible.
        with tc.tile_pool(name="ph1", bufs=1) as ph1, \
             tc.tile_pool(name="ph1p", bufs=2, space="PSUM") as ph1p:
            raw_v = ph1.tile([P, NT_Q, P], F32, tag="rawv", name="raw_v")
            rawT_v = ph1.tile([P, NT_Q, P], F32R, tag="rawTv", name="rawT_v")
            raw_q = ph1.tile([P, NT_Q, P], F32, tag="rawq", name="raw_q")
            rawT_q = ph1.tile([P, NT_Q, P], F32R, tag="rawTq", name="rawT_q")
            sides = (
                (value, v_actT, Wv_r, bv_t, "v", raw_v, rawT_v),
                (query, q_actT, Wq_r, bq_t, "q", raw_q, rawT_q),
            )
            # The whole v side comes first: pass-1 chunk 0 sweeps all 32
            # k-tiles (full v_actT + v_act) but needs only q chunk 0.
            for c in range(4):
                nc.sync.dma_start(
                    out=raw_v[:, ts(c, 8), :],
                    in_=value.rearrange("(t p) d -> p t d", p=P)[:, ts(c, 8), :])
            for c in range(4):
                nc.sync.dma_start(
                    out=raw_q[:, ts(c, 8), :],
                    in_=query.rearrange("(t p) d -> p t d", p=P)[:, ts(c, 8), :])

            def lin_chunk(c, dstT, W_r, b_t, nm, raw, rawT):
                # batch 4 transposes per PSUM->SBUF eviction, alternating the
                # eviction between DVE and ACT (both idle-ish in the ramp)
                for g in range(2):
                    t0 = c * 8 + g * 4
                    pst = ph1p.tile([P, 4, P], F32, tag="tp" + nm, name="pst")
                    for i in range(4):
                        nc.tensor.transpose(pst[:, i, :], raw[:, t0 + i, :],
                                            ident)
                    if g % 2 == 0:
                        nc.vector.tensor_copy(rawT[:, t0:t0 + 4, :], pst)
                    else:
                        nc.scalar.copy(rawT[:, t0:t0 + 4, :], pst)
                psl = ph1p.tile([P, 1024], F32, tag="lin" + nm, bufs=1,
                                name="psl")
                for h in range(2):
                    nc.tensor.matmul(
                        psl[:, ts(h, 512)], W_r,
                        rawT[:, c * 8 + h * 4:c * 8 + (h + 1) * 4, :]
                        .rearrange("d a b -> d (a b)"),
                        start=True, stop=True)
                nc.scalar.activation(dstT[:, ts(c, 1024)], psl, AF.Gelu,
                                     bias=b_t, scale=1.0)

            def vact_chunk(c):
                for g in range(2):
                    t0 = c * 8 + g * 4
                    pst = ph1p.tile([P, 4, P], F32, tag="tpv", name="pst2")
                    for i in range(4):
                        nc.tensor.transpose(
                            pst[:, i, :],
                            v_actT[:, ts(t0 + i, P)].bitcast(F32), ident)
                    if g % 2 == 0:
                        nc.vector.tensor_copy(v_act[:, t0:t0 + 4, :], pst)
                    else:
                        nc.scalar.copy(v_act[:, t0:t0 + 4, :], pst)

            for c in range(4):
                lin_chunk(c, v_actT, Wv_r, bv_t, "v", raw_v, rawT_v)
                # v_act natural orientation [k, h] lags one chunk so the PE
                # doesn't head-of-line block on the gelu just above.
                if c >= 1:
                    vact_chunk(c - 1)
            for c in range(4):
                lin_chunk(c, q_actT, Wq_r, bq_t, "q", raw_q, rawT_q)
                if c == 0:
                    vact_chunk(3)

        # ---------------- Phase 2: attention (both orientations) -----------
        build_ones()
        with tc.tile_pool(name="scp", bufs=2, space="PSUM") as scp, \
             tc.tile_pool(name="ctxp", bufs=1, space="PSUM") as ctxp, \
             tc.tile_pool(name="bcp", bufs=2, space="PSUM") as bcp, \
             tc.tile_pool(name="eTp", bufs=8) as eTp, \
             tc.tile_pool(name="emkp", bufs=2) as emkp, \
             tc.tile_pool(name="attnp", bufs=2) as attnp, \
             tc.tile_pool(name="invtp", bufs=2) as invtp, \
             tc.tile_pool(name="smallp", bufs=4) as smallp:
            for c in range(NCH):
                # ---- pass 1: scoreT [k, m-chunk] -> exp -> ctx accumulation
                ctx_ps = ctxp.tile([P, MC], F32, tag="ctx")
                for kt in range(NT_K):
                    s_ps = scp.tile([P, MC], F32, tag="sc")
                    for h in range(MC // 512):
                        nc.tensor.matmul(
                            s_ps[:, ts(h, 512)], v_actT[:, ts(kt, P)],
                            q_actT[:, c * MC + h * 512:c * MC + (h + 1) * 512],
                            start=True, stop=True)
                    eT = eTp.tile([P, MC], F32R, tag="eT")
                    nc.scalar.activation(eT, s_ps, AF.Exp)
                    for h in range(MC // 512):
                        nc.tensor.matmul(
                            ctx_ps[:, ts(h, 512)], v_act[:, kt, :],
                            eT[:, ts(h, 512)],
                            start=(kt == 0), stop=(kt == NT_K - 1))
                nc.vector.tensor_copy(ctxT[:, ts(c, MC)], ctx_ps)

                # ---- pass 2: score [m-tile, k] -> exp+sums -> attn out
                for t in range(c * (NT_Q // NCH), (c + 1) * (NT_Q // NCH)):
                    e_mk = emkp.tile([P, LV], F32, tag="emk")
                    parts = smallp.tile([P, NKC], F32, tag="parts")
                    for kc in range(NKC):
                        s2 = scp.tile([P, KC], F32, tag="sc")
                        for h in range(KC // 512):
                            nc.tensor.matmul(
                                s2[:, ts(h, 512)], q_actT[:, ts(t, P)],
                                v_actT[:, kc * KC + h * 512:kc * KC + (h + 1) * 512],
                                start=True, stop=True)
                        nc.scalar.activation(
                            e_mk[:, ts(kc, KC)], s2, AF.Exp,
                            accum_out=parts[:, kc:kc + 1])
                    sumt = smallp.tile([P, 1], F32, tag="sumt")
                    nc.vector.reduce_sum(sumt, parts, axis=AX.X,
                                         opt_input=False)
                    nc.vector.reciprocal(inv_sums[:, t:t + 1], sumt)
                    attn_t = attnp.tile([P, LV], F32, tag="attn")
                    nc.vector.tensor_scalar_mul(attn_t, e_mk,
                                                inv_sums[:, t:t + 1])
                    nc.sync.dma_start(out=attn_t_view[t], in_=attn_t)

                    # ---- incremental ctx normalization for this m-tile:
                    # inv_sums[:, t] --PE transpose--> [1, 128] row, then
                    # rank-1 matmul ones.T @ row broadcasts it across
                    # partitions; DVE multiplies ctxT against the broadcast
                    # straight from PSUM.
                    inv_ps = bcp.tile([1, P], F32, tag="bc", name="inv_ps")
                    nc.tensor.transpose(inv_ps, inv_sums[:, t:t + 1], ident)
                    invT_t = invtp.tile([1, P], F32R, tag="invT")
                    nc.vector.tensor_copy(invT_t, inv_ps)
                    bc_ps = bcp.tile([P, P], F32, tag="bc", name="bc_ps")
                    nc.tensor.matmul(bc_ps, ones_r, invT_t,
                                     start=True, stop=True)
                    nc.vector.tensor_mul(ctxn[:, ts(t, P)], ctxT[:, ts(t, P)],
                                         bc_ps)

        # ---------------- Phase 3: out = gelu(ctxn) @ Wo + bo ---------------
        # The p3 pool boundary doubles as the scheduler's fence that keeps
        # these gelus out of the exp stream (one act-table switch, not 20).
        with tc.tile_pool(name="p3", bufs=1) as p3, \
             tc.tile_pool(name="p3p", bufs=4, space="PSUM") as p3p:
            gctxT = p3.tile([P, LQ], F32R, tag="gctxT")
            out_view = out.rearrange("(t p) d -> p t d", p=P)
            for c in range(4):
                nc.scalar.activation(gctxT[:, ts(c, MC)], ctxn[:, ts(c, MC)],
                                     AF.Gelu)
                for t in range(c * 8, (c + 1) * 8):
                    ops = p3p.tile([P, D_OUT], F32, tag="op")
                    nc.tensor.matmul(ops, gctxT[:, ts(t, P)], Wo_r,
                                     start=True, stop=True)
                    nc.vector.tensor_add(out_sb[:, t, :], ops, bo_bc)
                nc.sync.dma_start(out=out_view[:, ts(c, 8), :],
                                  in_=out_sb[:, ts(c, 8), :])


_NC_CACHE = None


def _get_nc():
    global _NC_CACHE
    if _NC_CACHE is None:
        nc = bacc.Bacc("TRN2", debug=False)
        with tile.TileContext(nc) as tc:
            _body(tc)
        nc.compile()
        _NC_CACHE = nc
    return _NC_CACHE


def run(inputs, **spmd_kwargs):
    """Run on 8 NeuronCores; returns (out, attn, BassKernelResults)."""
    nc = _get_nc()
    f = lambda x: np.ascontiguousarray(np.asarray(x, dtype=np.float32))
    shared = {k: f(inputs[k]) for k in ("Wq", "bq", "Wv", "bv", "Wo", "bo")}
    in_maps = [
        {"query": f(inputs["query"][b]), "value": f(inputs["value"][b]),
         **shared}
        for b in range(B)
    ]
    res = run_bass_kernel_spmd(nc, in_maps, core_ids=list(range(B)),
                               **spmd_kwargs)
    out = np.stack([r["out"] for r in res.results])
    attn = np.stack([r["attn"] for r in res.results])
    return out, attn, res


def kernel(query, value, Wq, bq, Wv, bv, Wo, bo):
    out, attn, _ = run(dict(query=query, value=value, Wq=Wq, bq=bq,
                            Wv=Wv, bv=bv, Wo=Wo, bo=bo))
    return out, attn


# revision 80
# speedup vs baseline: 1.0030x; 1.0030x over previous
"""Trainium2 Bass kernel for nn_DotProductAttention_79929341378723.

Computation (per batch b, sharded batch-parallel over 8 NeuronCores):
    q   = gelu(query @ Wq + bq)          [LQ, H]
    v   = gelu(value @ Wv + bv)          [LV, H]
    s   = q @ v.T                        [LQ, LV]
    a   = softmax(s, axis=-1)            [LQ, LV]   (output)
    ctx = a @ v                          [LQ, H]
    out = gelu(ctx) @ Wo + bo            [LQ, D]    (output)

Design notes:
  - Scores are computed TWICE on the PE in the two orientations needed:
    scoreT [k, m] feeds exp -> expT tiles used directly as the moving operand
    of the ctx matmul (contraction k on partitions, no transposes of the
    16.7M-element attention matrix); score [m, k] feeds exp(+row sums via
    accum_out) -> normalized attn written straight out to HBM in its native
    layout (2MB contiguous DMAs).
  - All large matmuls run in float32r (TF32-like, 1 cycle/row at N=512);
    operands are rounded to f32r by their producing ACT/DVE instruction.
  - Softmax skips max-subtraction: |score| < ~30 for this distribution, so
    exp() cannot overflow fp32 and max-subtraction changes nothing
    numerically (it divides num/denom by the same constant).
  - The only cross-orientation data needed are the row sums: computed in the
    [m, k] pass via activation accum_out; broadcast along partitions for the
    ctx normalization via per-m-tile PE transpose + rank-1 ones matmul.
"""

import numpy as np
from contextlib import ExitStack

import concourse.bass as bass
import concourse.tile as tile
import concourse.mybir as mybir
from concourse import bacc
from concourse.bass import ts
from concourse.bass_utils import run_bass_kernel_spmd
from concourse.masks import make_identity

F32 = mybir.dt.float32
F32R = mybir.dt.float32r
AF = mybir.ActivationFunctionType
AX = mybir.AxisListType

B, LQ, LV = 8, 4096, 4096
D_OUT, D_HID = 128, 128
P = 128
NT_Q = LQ // P          # 32 query tiles
NT_K = LV // P          # 32 key/value tiles
MC = 1024               # pass-1 m-chunk width (exp free-dim)
NCH = LQ // MC          # 4 outer chunks
KC = 1024               # pass-2 k-chunk width (exp free-dim)
NKC = LV // KC          # 4 k-chunks per m-tile


def _body(tc):
    nc = tc.nc
    query = nc.dram_tensor("query", [LQ, D_OUT], F32, kind="ExternalInput").ap()
    value = nc.dram_tensor("value", [LV, D_HID], F32, kind="ExternalInput").ap()
    Wq = nc.dram_tensor("Wq", [D_OUT, D_HID], F32, kind="ExternalInput").ap()
    bq = nc.dram_tensor("bq", [D_HID], F32, kind="ExternalInput").ap()
    Wv = nc.dram_tensor("Wv", [D_HID, D_HID], F32, kind="ExternalInput").ap()
    bv = nc.dram_tensor("bv", [D_HID], F32, kind="ExternalInput").ap()
    Wo = nc.dram_tensor("Wo", [D_HID, D_OUT], F32, kind="ExternalInput").ap()
    bo = nc.dram_tensor("bo", [D_OUT], F32, kind="ExternalInput").ap()
    out = nc.dram_tensor("out", [LQ, D_OUT], F32, kind="ExternalOutput").ap()
    attn = nc.dram_tensor("attn", [LQ, LV], F32, kind="ExternalOutput").ap()

    attn_t_view = attn.rearrange("(t p) k -> t p k", p=P)

    with ExitStack() as ctx:
        const = ctx.enter_context(tc.tile_pool(name="const", bufs=1))
        stat = ctx.enter_context(tc.tile_pool(name="stat", bufs=1))

        ident = const.tile([P, P], F32, tag="ident")
        make_identity(nc, ident)

        # Weights: load fp32, round to f32r via DVE copy (matmul operands
        # must be written as f32r by their producer).
        wq_f = const.tile([P, P], F32, tag="wqf")
        wv_f = const.tile([P, P], F32, tag="wvf")
        wo_f = const.tile([P, P], F32, tag="wof")
        Wq_r = const.tile([P, P], F32R, tag="wq")
        Wv_r = const.tile([P, P], F32R, tag="wv")
        Wo_r = const.tile([P, P], F32R, tag="wo")
        bq_t = const.tile([P, 1], F32, tag="bq")
        bv_t = const.tile([P, 1], F32, tag="bv")
        warm = const.tile([P, 1], F32, tag="warm")

        def load_weights():
            # Emitted AFTER the first value-chunk DMAs: these five small
            # transfers otherwise occupy the queue ahead of the ramp's
            # critical first input chunk.
            nc.sync.dma_start(out=wv_f, in_=Wv)
            nc.sync.dma_start(out=wq_f, in_=Wq)
            nc.sync.dma_start(out=wo_f, in_=Wo)
            nc.sync.dma_start(out=bq_t, in_=bq.rearrange("(p o) -> p o", o=1))
            nc.sync.dma_start(out=bv_t, in_=bv.rearrange("(p o) -> p o", o=1))
            nc.vector.tensor_copy(Wv_r, wv_f)
            nc.vector.tensor_copy(Wq_r, wq_f)
            nc.vector.tensor_copy(Wo_r, wo_f)
            # prewarm the gelu act-table while the input DMAs stream
            nc.scalar.activation(warm, bq_t, AF.Gelu)
        # bo broadcast across partitions (DRE replication on SWDGE)
        bo_bc = const.tile([P, D_OUT], F32, tag="bo")
        bo_bcast_ap = bass.AP(tensor=bo.tensor, offset=bo.offset,
                              ap=[[0, P], [1, D_OUT]])
        nc.gpsimd.dma_start(out=bo_bc, in_=bo_bcast_ap)

        # Persistent activations (transposed layouts: [h, seq])
        q_actT = stat.tile([P, LQ], F32R, tag="q_actT")
        v_actT = stat.tile([P, LV], F32R, tag="v_actT")
        v_act = stat.tile([P, NT_K, D_HID], F32R, tag="v_act")  # [k, t, h]
        ctxT = stat.tile([P, LQ], F32, tag="ctxT")              # [h, m]
        ctxn = stat.tile([P, LQ], F32, tag="ctxn")              # normalized
        inv_sums = stat.tile([P, NT_Q], F32, tag="inv_sums")    # [m_in_tile, t]
        out_sb = stat.tile([P, NT_Q, D_OUT], F32, tag="out_sb")
        ones_f = stat.tile([1, P], F32, tag="ones_f")
        ones_r = stat.tile([1, P], F32R, tag="ones_r")

        def build_ones():
            nc.gpsimd.memset(ones_f, 1.0)
            nc.vector.tensor_copy(ones_r, ones_f)

        # ---------------- Phase 1: input transposes + q/v linears ----------
        # q and v interleave per-chunk so the first attention chunk's inputs
        # are ready as early as possible.
        with tc.tile_pool(name="ph1", bufs=1) as ph1, \
             tc.tile_pool(name="ph1p", bufs=2, space="PSUM") as ph1p:
            raw_v = ph1.tile([P, NT_Q, P], F32, tag="rawv", name="raw_v")
            rawT_v = ph1.tile([P, NT_Q, P], F32R, tag="rawTv", name="rawT_v")
            raw_q = ph1.tile([P, NT_Q, P], F32, tag="rawq", name="raw_q")
            rawT_q = ph1.tile([P, NT_Q, P], F32R, tag="rawTq", name="rawT_q")
            sides = (
                (value, v_actT, Wv_r, bv_t, "v", raw_v, rawT_v),
                (query, q_actT, Wq_r, bq_t, "q", raw_q, rawT_q),
            )
            # The whole v side comes first: pass-1 chunk 0 sweeps all 32
            # k-tiles (full v_actT + v_act) but needs only q chunk 0.
            for c in range(4):
                nc.sync.dma_start(
                    out=raw_v[:, ts(c, 8), :],
                    in_=value.rearrange("(t p) d -> p t d", p=P)[:, ts(c, 8), :])
            load_weights()
            for c in range(4):
                nc.sync.dma_start(
                    out=raw_q[:, ts(c, 8), :],
                    in_=query.rearrange("(t p) d -> p t d", p=P)[:, ts(c, 8), :])

            def lin_chunk(c, dstT, W_r, b_t, nm, raw, rawT):
                # batch 4 transposes per PSUM->SBUF eviction, alternating the
                # eviction between DVE and ACT (both idle-ish in the ramp)
                for g in range(2):
                    t0 = c * 8 + g * 4
                    pst = ph1p.tile([P, 4, P], F32, tag="tp" + nm, name="pst")
                    for i in range(4):
                        nc.tensor.transpose(pst[:, i, :], raw[:, t0 + i, :],
                                            ident)
                    if g % 2 == 0:
                        nc.vector.tensor_copy(rawT[:, t0:t0 + 4, :], pst)
                    else:
                        nc.scalar.copy(rawT[:, t0:t0 + 4, :], pst)
                psl = ph1p.tile([P, 1024], F32, tag="lin" + nm, bufs=1,
                                name="psl")
                for h in range(2):
                    nc.tensor.matmul(
                        psl[:, ts(h, 512)], W_r,
                        rawT[:, c * 8 + h * 4:c * 8 + (h + 1) * 4, :]
                        .rearrange("d a b -> d (a b)"),
                        start=True, stop=True)
                nc.scalar.activation(dstT[:, ts(c, 1024)], psl, AF.Gelu,
                                     bias=b_t, scale=1.0)

            def vact_chunk(c):
                for g in range(2):
                    t0 = c * 8 + g * 4
                    pst = ph1p.tile([P, 4, P], F32, tag="tpv", name="pst2")
                    for i in range(4):
                        nc.tensor.transpose(
                            pst[:, i, :],
                            v_actT[:, ts(t0 + i, P)].bitcast(F32), ident)
                    if g % 2 == 0:
                        nc.vector.tensor_copy(v_act[:, t0:t0 + 4, :], pst)
                    else:
                        nc.scalar.copy(v_act[:, t0:t0 + 4, :], pst)

            for c in range(4):
                lin_chunk(c, v_actT, Wv_r, bv_t, "v", raw_v, rawT_v)
                # v_act natural orientation [k, h] lags one chunk so the PE
                # doesn't head-of-line block on the gelu just above.
                if c >= 1:
                    vact_chunk(c - 1)
            for c in range(4):
                lin_chunk(c, q_actT, Wq_r, bq_t, "q", raw_q, rawT_q)
                if c == 0:
                    vact_chunk(3)

        # ---------------- Phase 2: attention (both orientations) -----------
        build_ones()
        with tc.tile_pool(name="scp", bufs=2, space="PSUM") as scp, \
             tc.tile_pool(name="ctxp", bufs=1, space="PSUM") as ctxp, \
             tc.tile_pool(name="bcp", bufs=2, space="PSUM") as bcp, \
             tc.tile_pool(name="eTp", bufs=8) as eTp, \
             tc.tile_pool(name="emkp", bufs=2) as emkp, \
             tc.tile_pool(name="attnp", bufs=2) as attnp, \
             tc.tile_pool(name="invtp", bufs=2) as invtp, \
             tc.tile_pool(name="smallp", bufs=4) as smallp:
            for c in range(NCH):
                # ---- pass 1: scoreT [k, m-chunk] -> exp -> ctx accumulation
                ctx_ps = ctxp.tile([P, MC], F32, tag="ctx")
                for kt in range(NT_K):
                    s_ps = scp.tile([P, MC], F32, tag="sc")
                    for h in range(MC // 512):
                        nc.tensor.matmul(
                            s_ps[:, ts(h, 512)], v_actT[:, ts(kt, P)],
                            q_actT[:, c * MC + h * 512:c * MC + (h + 1) * 512],
                            start=True, stop=True)
                    eT = eTp.tile([P, MC], F32R, tag="eT")
                    nc.scalar.activation(eT, s_ps, AF.Exp)
                    for h in range(MC // 512):
                        nc.tensor.matmul(
                            ctx_ps[:, ts(h, 512)], v_act[:, kt, :],
                            eT[:, ts(h, 512)],
                            start=(kt == 0), stop=(kt == NT_K - 1))
                nc.vector.tensor_copy(ctxT[:, ts(c, MC)], ctx_ps)

                # ---- pass 2: score [m-tile, k] -> exp+sums -> attn out
                for t in range(c * (NT_Q // NCH), (c + 1) * (NT_Q // NCH)):
                    e_mk = emkp.tile([P, LV], F32, tag="emk")
                    parts = smallp.tile([P, NKC], F32, tag="parts")
                    for kc in range(NKC):
                        s2 = scp.tile([P, KC], F32, tag="sc")
                        for h in range(KC // 512):
                            nc.tensor.matmul(
                                s2[:, ts(h, 512)], q_actT[:, ts(t, P)],
                                v_actT[:, kc * KC + h * 512:kc * KC + (h + 1) * 512],
                                start=True, stop=True)
                        nc.scalar.activation(
                            e_mk[:, ts(kc, KC)], s2, AF.Exp,
                            accum_out=parts[:, kc:kc + 1])
                    sumt = smallp.tile([P, 1], F32, tag="sumt")
                    nc.vector.reduce_sum(sumt, parts, axis=AX.X,
                                         opt_input=False)
                    nc.vector.reciprocal(inv_sums[:, t:t + 1], sumt)
                    attn_t = attnp.tile([P, LV], F32, tag="attn")
                    nc.vector.tensor_scalar_mul(attn_t, e_mk,
                                                inv_sums[:, t:t + 1])
                    nc.sync.dma_start(out=attn_t_view[t], in_=attn_t)

                    # ---- incremental ctx normalization for this m-tile:
                    # inv_sums[:, t] --PE transpose--> [1, 128] row, then
                    # rank-1 matmul ones.T @ row broadcasts it across
                    # partitions; DVE multiplies ctxT against the broadcast
                    # straight from PSUM.
                    inv_ps = bcp.tile([1, P], F32, tag="bc", name="inv_ps")
                    nc.tensor.transpose(inv_ps, inv_sums[:, t:t + 1], ident)
                    invT_t = invtp.tile([1, P], F32R, tag="invT")
                    nc.vector.tensor_copy(invT_t, inv_ps)
                    bc_ps = bcp.tile([P, P], F32, tag="bc", name="bc_ps")
                    nc.tensor.matmul(bc_ps, ones_r, invT_t,
                                     start=True, stop=True)
                    nc.vector.tensor_mul(ctxn[:, ts(t, P)], ctxT[:, ts(t, P)],
                                         bc_ps)

        # ---------------- Phase 3: out = gelu(ctxn) @ Wo + bo ---------------
        # The p3 pool boundary doubles as the scheduler's fence that keeps
        # these gelus out of the exp stream (one act-table switch, not 20).
        with tc.tile_pool(name="p3", bufs=1) as p3, \
             tc.tile_pool(name="p3p", bufs=4, space="PSUM") as p3p:
            gctxT = p3.tile([P, LQ], F32R, tag="gctxT")
            out_view = out.rearrange("(t p) d -> p t d", p=P)
            for c in range(4):
                nc.scalar.activation(gctxT[:, ts(c, MC)], ctxn[:, ts(c, MC)],
                                     AF.Gelu)
                for t in range(c * 8, (c + 1) * 8):
                    ops = p3p.tile([P, D_OUT], F32, tag="op")
                    nc.tensor.matmul(ops, gctxT[:, ts(t, P)], Wo_r,
                                     start=True, stop=True)
                    nc.vector.tensor_add(out_sb[:, t, :], ops, bo_bc)
                nc.sync.dma_start(out=out_view[:, ts(c, 8), :],
                                  in_=out_sb[:, ts(c, 8), :])


_NC_CACHE = None


def _get_nc():
    global _NC_CACHE
    if _NC_CACHE is None:
        nc = bacc.Bacc("TRN2", debug=False)
        with tile.TileContext(nc) as tc:
            _body(tc)
        nc.compile()
        _NC_CACHE = nc
    return _NC_CACHE


def run(inputs, **spmd_kwargs):
    """Run on 8 NeuronCores; returns (out, attn, BassKernelResults)."""
    nc = _get_nc()
    f = lambda x: np.ascontiguousarray(np.asarray(x, dtype=np.float32))
    shared = {k: f(inputs[k]) for k in ("Wq", "bq", "Wv", "bv", "Wo", "bo")}
    in_maps = [
        {"query": f(inputs["query"][b]), "value": f(inputs["value"][b]),
         **shared}
        for b in range(B)
    ]
    res = run_bass_kernel_spmd(nc, in_maps, core_ids=list(range(B)),
                               **spmd_kwargs)
    out = np.stack([r["out"] for r in res.results])
    attn = np.stack([r["attn"] for r in res.results])
    return out, attn, res


def kernel(query, value, Wq, bq, Wv, bv, Wo, bo):
    out, attn, _ = run(dict(query=query, value=value, Wq=Wq, bq=bq,
                            Wv=Wv, bv=bv, Wo=Wo, bo=bo))
    return out, attn
